# revision 20
# baseline (speedup 1.0000x reference)
"""Trainium2 Bass kernel for nn_EncoderLayer_88476326298146 (sparse graph attention).

Row-sharded across 8 NeuronCores: core c owns nodes [c*2048, (c+1)*2048) and the
edges targeting them (host-sorted by row into 16 windows of 128 rows, padded to a
fixed per-window count TW). k/v (bf16, [-k|v] rows, k negated) are replicated via
AllGather; per-edge col features come from dma_gather.

Engine-balanced v2:
- LN affine (g,b) folded into the following weight matrices on host; LN stats via
  bn_stats/bn_aggr; biases applied via rank-1 ones-row matmuls on the PE.
- diff = q_row - k_col computed on the PE (s2^T@q then accumulate identity@(-k));
  square on the Act engine from PSUM; per-head reduce on DVE.
- exp emitted as bf16 PAIRS so the alpha*v multiply runs in DVE 2x mode.
- segment softmax with m=0 (scores <= max pos_att_bias, exp bounded); segment sums
  via host-built one-hot matrices on the PE.
- FFN1 produced transposed (lhsT=W1 tiles, rhs=z2^T) with gelu+bias fused on Act;
  FFN2 consumes h^T directly as lhsT: zero transposes in the FFN.
- All intermediates (x, x2, z^T, q, h^T) stay in SBUF; only k/v round-trips DRAM
  for the AllGather + gather.
"""
import os
import numpy as np

import concourse.bass as bass
import concourse.bacc as bacc
import concourse.mybir as mybir
import concourse.tile as tile
from concourse.bass_utils import run_bass_kernel_spmd
from concourse.library_config import mlp as mlp_lib

L, E, SP, C, H, DH, HID = 16384, 131072, 20000, 512, 8, 64, 1024
NCORES = 8
RS = L // NCORES
NW = RS // 128
P = 128
F32 = mybir.dt.float32
BF16 = mybir.dt.bfloat16
I16 = mybir.dt.int16
FP8 = mybir.dt.float8e4
WS = 64.0  # weight pre-scale (fp8 subnormal avoidance); descaled in Act casts
AF = mybir.ActivationFunctionType
ALU = mybir.AluOpType
AX = mybir.AxisListType

_cache = {}
_SKIP = set(os.environ.get("KSKIP", "").split(","))


def _build(TW):
    NT = TW // P
    GS = 3  # tiles per score group (PSUM: GS banks for qe)
    inv_s = 1.0 / float(np.sqrt(DH))
    nc = bacc.Bacc("TRN2", target_bir_lowering=False, debug=False, num_devices=NCORES)

    x_in = nc.dram_tensor("x_in", [RS, C], F32, kind="ExternalInput")
    w_qkv = nc.dram_tensor("w_qkv", [C, 3 * C], FP8, kind="ExternalInput")
    w_o = nc.dram_tensor("w_o", [C, C], FP8, kind="ExternalInput")
    w_1 = nc.dram_tensor("w_1", [C, HID], FP8, kind="ExternalInput")
    w_2 = nc.dram_tensor("w_2", [HID, C], FP8, kind="ExternalInput")
    w_vec4 = nc.dram_tensor("w_vec4", [32, C], BF16, kind="ExternalInput")
    b1_col = nc.dram_tensor("b1_col", [P, HID // P], F32, kind="ExternalInput")
    brows = nc.dram_tensor("brows", [1, 4 * C], FP8, kind="ExternalInput")
    ident_in = nc.dram_tensor("ident_in", [P, P], BF16, kind="ExternalInput")
    ones_in = nc.dram_tensor("ones_in", [1, P], FP8, kind="ExternalInput")
    eps_in = nc.dram_tensor("eps_in", [P, 1], F32, kind="ExternalInput")
    eidx = nc.dram_tensor("eidx", [NW, P, TW // 16], I16, kind="ExternalInput")
    rel4 = nc.dram_tensor("rel4", [NW, P, NT, 4], BF16, kind="ExternalInput")
    biasC = nc.dram_tensor("biasC", [NW, P, NT, 8], F32, kind="ExternalInput")
    s_e2r = nc.dram_tensor("s_e2r", [NW, P, NT, P], BF16, kind="ExternalInput")
    s_r2e = nc.dram_tensor("s_r2e", [NW, P, NT, P], BF16, kind="ExternalInput")
    y_out = nc.dram_tensor("y_out", [RS, C], F32, kind="ExternalOutput")
    dbg_out = nc.dram_tensor("dbg_out", [P, NW, HID], BF16, kind="ExternalOutput")
    dbg2_out = nc.dram_tensor("dbg2_out", [P, 2048], F32, kind="ExternalOutput")

    x_t = x_in.ap().rearrange("(m p) n -> p m n", p=P)
    y_t = y_out.ap().rearrange("(m p) n -> p m n", p=P)

    with tile.TileContext(nc) as tc:
        with tc.tile_pool(name="dram", bufs=1, space="DRAM") as dram, \
             tc.tile_pool(name="const", bufs=1) as const:
            nc.gpsimd.load_library(mlp_lib)

            ident = const.tile([P, P], BF16)
            nc.sync.dma_start(ident[:], ident_in.ap())
            ones_s = const.tile([1, P], FP8)
            nc.sync.dma_start(ones_s[:], ones_in.ap())
            eps_t = const.tile([P, 1], F32)
            nc.sync.dma_start(eps_t[:], eps_in.ap())
            brow_s = const.tile([1, 4 * C], FP8)
            nc.sync.dma_start(brow_s[:], brows.ap())
            wvec_s = const.tile([32, C], BF16)
            nc.sync.dma_start(wvec_s[:], w_vec4.ap())
            b1c_s = const.tile([P, HID // P], F32)
            nc.sync.dma_start(b1c_s[:], b1_col.ap())

            x_all = const.tile([P, NW, C], F32)
            x2_all = const.tile([P, NW, C], BF16)
            zt_sbuf = const.tile([P, C // P, RS], FP8)
            q_sbuf = const.tile([P, NW, C], BF16)
            hT_sbuf = const.tile([P, HID // P, RS], FP8)

            kv_shard = dram.tile([RS, 2 * C], BF16)
            if "ag" not in _SKIP:
                kvt = dram.tile([L, 2 * C], BF16, addr_space="Shared")
            else:
                kvt = dram.tile([L, 2 * C], BF16)

            # ---------- LN helper: stats + normalized bf16 z (no affine) ----
            def ln_win(wk, ptp, src, m, copy_eng="v"):
                stats = wk.tile([P, 6], F32, tag="ln_st")
                nc.vector.bn_stats(stats[:], src)
                mv = wk.tile([P, 2], F32, tag="ln_mv")
                nc.vector.bn_aggr(mv[:], stats[:])
                sd = wk.tile([P, 1], F32, tag="ln_sd")
                nc.scalar.activation(sd[:], mv[:, 1:2], AF.Sqrt, bias=eps_t[:], scale=1.0)
                rs_ = wk.tile([P, 1], F32, tag="ln_rs")
                nc.vector.reciprocal(rs_[:], sd[:])
                nmr = wk.tile([P, 1], F32, tag="ln_nmr")
                nc.vector.scalar_tensor_tensor(
                    out=nmr[:], in0=mv[:, 0:1], scalar=-1.0, in1=rs_[:],
                    op0=ALU.mult, op1=ALU.mult)
                zf = wk.tile([P, C], BF16, tag="ln_zf")
                nc.scalar.activation(zf[:], src, AF.Identity, bias=nmr[:], scale=rs_[:])
                tp = ptp.tile([P, C], BF16, tag="tp")
                for c4 in range(C // P):
                    nc.tensor.transpose(tp[:, c4 * P:(c4 + 1) * P],
                                        zf[:, c4 * P:(c4 + 1) * P], ident[:])
                if copy_eng == "a":
                    nc.scalar.activation(
                        zt_sbuf[:, :, m * P:(m + 1) * P],
                        tp[:].rearrange("p (c4 q) -> p c4 q", q=P), AF.Identity)
                else:
                    nc.vector.tensor_copy(
                        zt_sbuf[:, :, m * P:(m + 1) * P],
                        tp[:].rearrange("p (c4 q) -> p c4 q", q=P))

            # ============ P1: LN1 + QKV ============
            if "p1" not in _SKIP:
             with tc.tile_pool(name="p1", bufs=2) as wk, \
                 tc.tile_pool(name="p1c", bufs=1) as cst, \
                 tc.tile_pool(name="p1tp", bufs=2, space="PSUM") as ptp, \
                 tc.tile_pool(name="p1ps", bufs=2, space="PSUM") as pps:
                wqkv_s = cst.tile([P, C // P, 3 * C], FP8, name="wqkv")
                nc.gpsimd.dma_start(wqkv_s[:], w_qkv.ap().rearrange("(ko p) n -> p ko n", p=P))
                kv_sh_t = kv_shard[:].rearrange("(m p) n -> p m n", p=P)
                for m in range(NW):
                    nc.sync.dma_start(x_all[:, m, :], x_t[:, m, :])
                    ln_win(wk, ptp, x_all[:, m, :], m, copy_eng="a")
                    for nb in range(3):
                        ps = pps.tile([P, C], F32, tag="ps")
                        for ko in range(0, C // P, 2):
                            nc.tensor.matmul(
                                ps[:],
                                lhsT=zt_sbuf[:, ko:ko + 2, m * P:(m + 1) * P],
                                rhs=wqkv_s[:, ko:ko + 2, nb * C:(nb + 1) * C],
                                start=(ko == 0), stop=(ko == C // P - 2 and nb != 0),
                                perf_mode=mybir.MatmulPerfMode.DoubleRow)
                        if nb == 0:
                            nc.tensor.matmul(ps[:], lhsT=ones_s[:], rhs=brow_s[0:1, 0:C],
                                             start=False, stop=True)
                            nc.scalar.activation(q_sbuf[:, m, :], ps[:], AF.Identity,
                                                 scale=1.0 / WS)
                        else:
                            kvb = wk.tile([P, C], BF16, tag="kvb")
                            # k stored negated so the edge phase can accumulate
                            # qe + (-k) on the PE via an identity matmul.
                            nc.vector.tensor_scalar_mul(
                                kvb[:], ps[:], (-1.0 if nb == 1 else 1.0) / WS)
                            nc.sync.dma_start(
                                kv_sh_t[:, m, (nb - 1) * C:nb * C], kvb[:])

            # ============ P2: AllGather ============
            if "ag" not in _SKIP:
                nc.gpsimd.collective_compute(
                    "AllGather", ALU.bypass, replica_groups=[list(range(NCORES))],
                    ins=[kv_shard[:].opt()], outs=[kvt[:].opt()])

            # ============ P3: edge windows + Wo + residual ============
            if "edge" not in _SKIP:
             with tc.tile_pool(name="big", bufs=2) as big, \
                 tc.tile_pool(name="ew", bufs=2) as wk, \
                 tc.tile_pool(name="pqe", bufs=1, space="PSUM") as pqe, \
                 tc.tile_pool(name="ppsW", bufs=1, space="PSUM") as ppsW, \
                 tc.tile_pool(name="ptpc", bufs=1, space="PSUM") as ptpc, \
                 tc.tile_pool(name="p5ps", bufs=2, space="PSUM") as p5ps, \
                 tc.tile_pool(name="woc", bufs=1) as woc:
                wo_s = woc.tile([P, C // P, C], FP8, name="wo")
                nc.gpsimd.dma_start(wo_s[:], w_o.ap().rearrange("(ko p) n -> p ko n", p=P))
                for w in range(NW):
                    idx_t = big.tile([P, TW // 16], I16, tag="idx", bufs=3)
                    nc.gpsimd.dma_start(idx_t[:], eidx.ap()[w])
                    kv_g = big.tile([P, NT, 2 * C], BF16, tag="kv", bufs=3)
                    nc.gpsimd.dma_gather(
                        out_ap=kv_g[:], in_ap=kvt[:], idxs_ap=idx_t[:],
                        num_idxs=TW, num_idxs_reg=TW, elem_size=2 * C,
                        single_packet=False)
                    s1_t = big.tile([P, NT, P], BF16, tag="s1")
                    nc.sync.dma_start(s1_t[:], s_e2r.ap()[w])
                    s2_t = big.tile([P, NT, P], BF16, tag="s2")
                    nc.scalar.dma_start(s2_t[:], s_r2e.ap()[w])
                    rel_t = big.tile([P, NT, 4], BF16, tag="rel")
                    nc.sync.dma_start(rel_t[:], rel4.ap()[w])
                    bias_t = big.tile([P, NT, 8], F32, tag="bias")
                    nc.sync.dma_start(bias_t[:], biasC.ap()[w])

                    psW = ppsW.tile([P, 560], F32, tag="psW")
                    for t0 in range(0, NT, GS):
                        tb = min(GS, NT - t0)
                        qe = pqe.tile([P, GS, C], F32, tag="qe")
                        for d_ in range(tb):
                            nc.tensor.matmul(qe[:, d_, :], lhsT=s2_t[:, t0 + d_, :],
                                             rhs=q_sbuf[:, w, :], start=True, stop=False)
                            nc.tensor.matmul(qe[:, d_, :], lhsT=ident[:],
                                             rhs=kv_g[:, t0 + d_, 0:C],
                                             start=False, stop=True)
                        dsq = wk.tile([P, GS, C], BF16, tag="dsq")
                        nc.scalar.activation(dsq[:, 0:tb, :], qe[:, 0:tb, :], AF.Square)
                        s8 = wk.tile([P, GS, H], F32, tag="s8")
                        nc.vector.reduce_sum(
                            s8[:, 0:tb, :],
                            dsq[:, 0:tb, :].rearrange("p t (h d) -> p t h d", h=H),
                            axis=AX.X)
                        sc = wk.tile([P, GS, H], F32, tag="sc")
                        nc.vector.scalar_tensor_tensor(
                            out=sc[:, 0:tb, :], in0=s8[:, 0:tb, :], scalar=-inv_s,
                            in1=bias_t[:, t0:t0 + tb, :], op0=ALU.mult, op1=ALU.add)
                        eaux = wk.tile([P, GS, 48], BF16, tag="eaux")
                        nc.scalar.activation(
                            eaux[:, 0:tb, 0:16].rearrange("p t (h j) -> p t h j", h=H),
                            sc[:, 0:tb, :].unsqueeze(3).broadcast_to([P, tb, H, 2]),
                            AF.Exp)
                        exp2 = wk.tile([P, GS, 16], BF16, tag="exp2")
                        nc.scalar.activation(
                            exp2[:, 0:tb, :].rearrange("p t (h j) -> p t h j", h=H),
                            sc[:, 0:tb, :].unsqueeze(3).broadcast_to([P, tb, H, 2]),
                            AF.Exp)
                        if w == 0 and t0 == 0 and os.environ.get("KDBG") == "edge1":
                            d2 = wk.tile([P, 2048], F32, tag="d2")
                            nc.vector.tensor_copy(d2[:, 0:24], s8[:].rearrange("p t h -> p (t h)"))
                            nc.vector.tensor_copy(d2[:, 24:48], sc[:].rearrange("p t h -> p (t h)"))
                            nc.vector.tensor_copy(d2[:, 48:560], qe[:, 0, :])
                            nc.vector.tensor_copy(d2[:, 560:584], bias_t[:, 0:3, :].rearrange("p t h -> p (t h)"))
                            nc.sync.dma_start(dbg2_out.ap(), d2[:])
                        pev = wk.tile([P, GS, C], BF16, tag="pev")
                        for d_ in range(tb):
                            nc.vector.tensor_mul(
                                pev[:, d_, :].rearrange("p (h a j) -> p h a j", h=H, j=2),
                                exp2[:, d_, :].rearrange("p (h j) -> p h j", h=H)
                                    .unsqueeze(2).broadcast_to([P, H, DH // 2, 2]),
                                kv_g[:, t0 + d_, C:2 * C]
                                    .rearrange("p (h a j) -> p h a j", h=H, j=2))
                        nc.vector.tensor_mul(
                            eaux[:, 0:tb, 16:48].rearrange("p t (h a) -> p t h a", h=H),
                            eaux[:, 0:tb, 0:16].rearrange("p t (h j) -> p t h j", h=H)[:, :, :, 0:1]
                                .broadcast_to([P, tb, H, 4]),
                            rel_t[:, t0:t0 + tb, :].unsqueeze(2)
                                .broadcast_to([P, tb, H, 4]))
                        for d_ in range(tb):
                            t = t0 + d_
                            nc.tensor.matmul(psW[:, 0:512], lhsT=s1_t[:, t, :],
                                             rhs=pev[:, d_, :],
                                             start=(t == 0), stop=False)
                            nc.tensor.matmul(psW[:, 512:560], lhsT=s1_t[:, t, :],
                                             rhs=eaux[:, d_, :],
                                             start=(t == 0), stop=(t == NT - 1))

                    den = wk.tile([P, 16], F32, tag="den")
                    nc.vector.tensor_scalar_max(den[:], psW[:, 512:528], 1e-30)
                    rden = wk.tile([P, 16], F32, tag="rden")
                    nc.vector.reciprocal(rden[:], den[:])
                    # fold the (unnormalized) Wvec term into psW[0:512]: w_vec4 is
                    # head-block-diagonal, so the per-(row,head) rden factors
                    # through the sum.
                    anr = wk.tile([P, 32], BF16, tag="anr")
                    nc.scalar.activation(anr[:], psW[:, 528:560], AF.Identity)
                    tpc = ptpc.tile([P, C], BF16, tag="tpc")
                    nc.tensor.transpose(tpc[0:32, 0:P], anr[:], ident[:])
                    an_ts = wk.tile([32, P], BF16, tag="an_ts")
                    nc.scalar.activation(an_ts[:], tpc[0:32, 0:P], AF.Identity)
                    nc.tensor.matmul(psW[:, 0:512], lhsT=an_ts[:], rhs=wvec_s[:],
                                     start=False, stop=True)
                    attin = wk.tile([P, C], BF16, tag="attin")
                    nc.vector.tensor_mul(
                        attin[:].rearrange("p (h d) -> p h d", h=H),
                        psW[:, 0:512].rearrange("p (h d) -> p h d", h=H),
                        rden[:].rearrange("p (h j) -> p h j", h=H)[:, :, 0:1]
                            .broadcast_to([P, H, DH]))
                    tpa = ptpc.tile([P, C], BF16, tag="tpc")
                    for c4 in range(C // P):
                        nc.tensor.transpose(tpa[:, c4 * P:(c4 + 1) * P],
                                            attin[:, c4 * P:(c4 + 1) * P], ident[:])
                    at_sb = wk.tile([P, C // P, P], FP8, tag="at_sb")
                    nc.scalar.activation(
                        at_sb[:], tpa[:].rearrange("p (c4 q) -> p c4 q", q=P),
                        AF.Identity)
                    x2ps = p5ps.tile([P, C], F32, tag="p5")
                    for ko in range(0, C // P, 2):
                        nc.tensor.matmul(x2ps[:], lhsT=at_sb[:, ko:ko + 2, :],
                                         rhs=wo_s[:, ko:ko + 2, :],
                                         start=(ko == 0), stop=False,
                                         perf_mode=mybir.MatmulPerfMode.DoubleRow)
                    nc.tensor.matmul(x2ps[:], lhsT=ones_s[:], rhs=brow_s[0:1, C:2 * C],
                                     start=False, stop=True)
                    nc.vector.scalar_tensor_tensor(
                        out=x2_all[:, w, :], in0=x2ps[:], scalar=1.0 / WS,
                        in1=x_all[:, w, :], op0=ALU.mult, op1=ALU.add)

            # ============ P4: LN2 + FFN (fused, per row-chunk) ============
            if "p4" not in _SKIP:
             with tc.tile_pool(name="p4", bufs=2) as wk, \
                 tc.tile_pool(name="p4c", bufs=1) as cst, \
                 tc.tile_pool(name="p4tp", bufs=2, space="PSUM") as ptp, \
                 tc.tile_pool(name="f1ps", bufs=2, space="PSUM") as pps1, \
                 tc.tile_pool(name="f2ps", bufs=2, space="PSUM") as pps2:
                w1_s = cst.tile([P, C // P, HID], FP8, name="w1")
                nc.gpsimd.dma_start(w1_s[:], w_1.ap().rearrange("(ko p) n -> p ko n", p=P))
                w2_s = cst.tile([P, HID // P, C], FP8, name="w2")
                nc.gpsimd.dma_start(w2_s[:], w_2.ap().rearrange("(ko p) n -> p ko n", p=P))
                for rc in range(RS // 512):
                    for m in range(rc * 4, rc * 4 + 4):
                        ln_win(wk, ptp, x2_all[:, m, :], m)
                    for ht in range(HID // P):
                        ps = pps1.tile([P, 512], F32, tag="ps1")
                        for ko in range(0, C // P, 2):
                            nc.tensor.matmul(
                                ps[:], lhsT=w1_s[:, ko:ko + 2, ht * P:(ht + 1) * P],
                                rhs=zt_sbuf[:, ko:ko + 2, rc * 512:(rc + 1) * 512],
                                start=(ko == 0), stop=(ko == C // P - 2),
                                perf_mode=mybir.MatmulPerfMode.DoubleRow)
                        nc.scalar.activation(
                            hT_sbuf[:, ht, rc * 512:(rc + 1) * 512], ps[:],
                            AF.Gelu_apprx_tanh, bias=b1c_s[:, ht:ht + 1], scale=1.0 / WS)
                    for m in range(rc * 4, rc * 4 + 4):
                        ps = pps2.tile([P, C], F32, tag="ps2")
                        for ht in range(0, HID // P, 2):
                            nc.tensor.matmul(ps[:], lhsT=hT_sbuf[:, ht:ht + 2, m * P:(m + 1) * P],
                                             rhs=w2_s[:, ht:ht + 2, :],
                                             start=(ht == 0), stop=False,
                                             perf_mode=mybir.MatmulPerfMode.DoubleRow)
                        nc.tensor.matmul(ps[:], lhsT=ones_s[:], rhs=brow_s[0:1, 2 * C:3 * C],
                                         start=False, stop=True)
                        yt = wk.tile([P, C], F32, tag="y")
                        nc.vector.scalar_tensor_tensor(
                            out=yt[:], in0=ps[:], scalar=1.0 / WS, in1=x2_all[:, m, :],
                            op0=ALU.mult, op1=ALU.add)
                        nc.sync.dma_start(y_t[:, m, :], yt[:])

    nc.compile()
    return nc


def _prep(inputs):
    row = np.asarray(inputs["row_index"]).astype(np.int64).ravel()
    col = np.asarray(inputs["col_index"]).astype(np.int64).ravel()
    tcol = np.asarray(inputs["to_col_index"]).astype(np.int64).ravel()
    bias = np.asarray(inputs["pos_att_bias"], dtype=np.float32)
    dist = np.asarray(inputs["dist"], dtype=np.float32).ravel()
    pos = np.asarray(inputs["pos"], dtype=np.float32)
    cpos = np.asarray(inputs["col_pos"], dtype=np.float32)

    order = np.argsort(row, kind="stable")
    rs_, cs_, ts_ = row[order], col[order], tcol[order]
    win = rs_ // P
    counts = np.bincount(win, minlength=L // P)
    TW = int(np.ceil(max(int(counts.max()), 1) / P) * P)
    NT = TW // P
    starts = np.zeros(L // P + 1, np.int64)
    np.cumsum(counts, out=starts[1:])

    eidx_h = np.zeros((NCORES, NW, P, TW // 16), np.int16)
    rel4_h = np.zeros((NCORES, NW, P, NT, 4), np.float32)
    bias_h = np.full((NCORES, NW, P, NT, 8), -1e4, np.float32)
    s1_h = np.zeros((NCORES, NW, P, NT, P), np.float32)
    s2_h = np.zeros((NCORES, NW, P, NT, P), np.float32)

    for gw in range(L // P):
        c, w = divmod(gw, NW)
        s, e = int(starts[gw]), int(starts[gw + 1])
        n = e - s
        if n == 0:
            continue
        ecols = cs_[s:e]
        erows = (rs_[s:e] - gw * P).astype(np.int64)
        eo = order[s:e]
        j = np.arange(n)
        wrap = np.zeros((16, TW // 16), np.int16)
        wrap[j % 16, j // 16] = ecols.astype(np.int16)
        eidx_h[c, w] = np.tile(wrap, (8, 1))
        t_of = j // P
        e_of = j % P
        rel4_h[c, w, e_of, t_of, 0:3] = (cpos[ts_[s:e]] - pos[rs_[s:e]]) / dist[eo][:, None]
        rel4_h[c, w, e_of, t_of, 3] = 1.0
        bias_h[c, w, e_of, t_of, :] = bias[eo]
        s1_h[c, w, e_of, t_of, erows] = 1.0
        s2_h[c, w, erows, t_of, e_of] = 1.0

    import ml_dtypes
    bf = ml_dtypes.bfloat16
    return (TW, eidx_h, rel4_h.astype(bf), bias_h,
            s1_h.astype(bf), s2_h.astype(bf))


def kernel(**inputs):
    import ml_dtypes
    bf = ml_dtypes.bfloat16
    x = np.asarray(inputs["x"], dtype=np.float32)
    TW, eidx_h, rel4_h, bias_h, s1_h, s2_h = _prep(inputs)
    if TW not in _cache:
        _cache[TW] = _build(TW)
    nc = _cache[TW]

    f32 = lambda k: np.asarray(inputs[k], np.float32)
    g1, b1l = f32("ln1_g"), f32("ln1_b")
    g2, b2l = f32("ln2_g"), f32("ln2_b")
    Wq, Wk, Wv, Wo = f32("Wq"), f32("Wk"), f32("Wv"), f32("Wo")
    # Fold LN affine into the following matmuls; fold bk into bq (only the
    # difference q-k matters) and bv into bo (sum_e alpha = 1 per head).
    Wq_, Wk_, Wv_ = g1[:, None] * Wq, g1[:, None] * Wk, g1[:, None] * Wv
    bq_ = (b1l @ Wq + f32("bq")) - (b1l @ Wk + f32("bk"))
    bo_ = (b1l @ Wv + f32("bv")) @ Wo + f32("bo")
    W1_ = g2[:, None] * f32("W1")
    b1_ = b2l @ f32("W1") + f32("b1")
    import ml_dtypes as _md
    f8 = _md.float8_e4m3
    WS = 64.0
    w_qkv = (np.concatenate([Wq_, Wk_, Wv_], axis=1) * WS).astype(f8)

    wv4 = np.concatenate([f32("Wvec"), f32("bvec")[None, :]], axis=0)
    w_vec4 = np.zeros((32, C), np.float32)
    for h in range(H):
        w_vec4[4 * h:4 * h + 4, h * DH:(h + 1) * DH] = wv4[:, h * DH:(h + 1) * DH]

    brows = np.zeros((1, 4 * C), np.float32)
    brows[0, 0:C] = bq_
    brows[0, C:2 * C] = bo_
    brows[0, 2 * C:3 * C] = f32("b2")
    b1_col = np.ascontiguousarray(b1_.reshape(HID // P, P).T)

    in_maps = []
    for c in range(NCORES):
        in_maps.append(dict(
            x_in=np.ascontiguousarray(x[c * RS:(c + 1) * RS]),
            w_qkv=w_qkv, w_o=(Wo * WS).astype(f8),
            w_1=(W1_ * WS).astype(f8), w_2=(f32("W2") * WS).astype(f8),
            w_vec4=w_vec4.astype(bf), b1_col=b1_col,
            brows=(brows * WS).astype(f8),
            ident_in=np.eye(P, dtype=np.float32).astype(bf),
            ones_in=np.ones((1, P), np.float32).astype(f8),
            eps_in=np.full((P, 1), 1e-5, np.float32),
            eidx=eidx_h[c], rel4=rel4_h[c], biasC=bias_h[c],
            s_e2r=s1_h[c], s_r2e=s2_h[c],
        ))
    _last["nc"] = nc
    _last["in_maps"] = in_maps
    res = run_bass_kernel_spmd(nc, in_maps, list(range(NCORES)))
    global _last_res
    _last_res = res
    y = np.concatenate([res.results[c]["y_out"] for c in range(NCORES)], axis=0)
    return np.asarray(y, np.float32)


_last = {}
_last_res = None


# revision 21
# speedup vs baseline: 1.0295x; 1.0295x over previous
"""Trainium2 Bass kernel for nn_EncoderLayer_88476326298146 (sparse graph attention).

Row-sharded across 8 NeuronCores: core c owns nodes [c*2048, (c+1)*2048) and the
edges targeting them (host-sorted by row into 16 windows of 128 rows, padded to a
fixed per-window count TW). k/v (bf16, [-k|v] rows, k negated) are replicated via
AllGather; per-edge col features come from dma_gather.

Engine-balanced v2:
- LN affine (g,b) folded into the following weight matrices on host; LN stats via
  bn_stats/bn_aggr; biases applied via rank-1 ones-row matmuls on the PE.
- diff = q_row - k_col computed on the PE (s2^T@q then accumulate identity@(-k));
  square on the Act engine from PSUM; per-head reduce on DVE.
- exp emitted as bf16 PAIRS so the alpha*v multiply runs in DVE 2x mode.
- segment softmax with m=0 (scores <= max pos_att_bias, exp bounded); segment sums
  via host-built one-hot matrices on the PE.
- FFN1 produced transposed (lhsT=W1 tiles, rhs=z2^T) with gelu+bias fused on Act;
  FFN2 consumes h^T directly as lhsT: zero transposes in the FFN.
- All intermediates (x, x2, z^T, q, h^T) stay in SBUF; only k/v round-trips DRAM
  for the AllGather + gather.
"""
import os
import numpy as np

import concourse.bass as bass
import concourse.bacc as bacc
import concourse.mybir as mybir
import concourse.tile as tile
from concourse.bass_utils import run_bass_kernel_spmd
from concourse.library_config import mlp as mlp_lib

L, E, SP, C, H, DH, HID = 16384, 131072, 20000, 512, 8, 64, 1024
NCORES = 8
RS = L // NCORES
NW = RS // 128
P = 128
F32 = mybir.dt.float32
BF16 = mybir.dt.bfloat16
I16 = mybir.dt.int16
FP8 = mybir.dt.float8e4
WS = 64.0  # weight pre-scale (fp8 subnormal avoidance); descaled in Act casts
AF = mybir.ActivationFunctionType
ALU = mybir.AluOpType
AX = mybir.AxisListType

_cache = {}
_SKIP = set(os.environ.get("KSKIP", "").split(","))


def _build(TW):
    NT = TW // P
    GS = 3  # tiles per score group (PSUM: GS banks for qe)
    inv_s = 1.0 / float(np.sqrt(DH))
    nc = bacc.Bacc("TRN2", target_bir_lowering=False, debug=False, num_devices=NCORES)

    x_in = nc.dram_tensor("x_in", [RS, C], F32, kind="ExternalInput")
    w_qkv = nc.dram_tensor("w_qkv", [C, 3 * C], FP8, kind="ExternalInput")
    w_o = nc.dram_tensor("w_o", [C, C], FP8, kind="ExternalInput")
    w_1 = nc.dram_tensor("w_1", [C, HID], FP8, kind="ExternalInput")
    w_2 = nc.dram_tensor("w_2", [HID, C], FP8, kind="ExternalInput")
    w_vec4 = nc.dram_tensor("w_vec4", [32, C], BF16, kind="ExternalInput")
    b1_col = nc.dram_tensor("b1_col", [P, HID // P], F32, kind="ExternalInput")
    brows = nc.dram_tensor("brows", [1, 4 * C], FP8, kind="ExternalInput")
    ident_in = nc.dram_tensor("ident_in", [P, P], BF16, kind="ExternalInput")
    ones_in = nc.dram_tensor("ones_in", [1, P], FP8, kind="ExternalInput")
    eps_in = nc.dram_tensor("eps_in", [P, 1], F32, kind="ExternalInput")
    eidx = nc.dram_tensor("eidx", [NW, P, TW // 16], I16, kind="ExternalInput")
    rel4 = nc.dram_tensor("rel4", [NW, P, NT, 4], BF16, kind="ExternalInput")
    biasC = nc.dram_tensor("biasC", [NW, P, NT, 8], F32, kind="ExternalInput")
    s_e2r = nc.dram_tensor("s_e2r", [NW, P, NT, P], BF16, kind="ExternalInput")
    s_r2e = nc.dram_tensor("s_r2e", [NW, P, NT, P], BF16, kind="ExternalInput")
    y_out = nc.dram_tensor("y_out", [RS, C], F32, kind="ExternalOutput")
    dbg_out = nc.dram_tensor("dbg_out", [P, NW, HID], BF16, kind="ExternalOutput")
    dbg2_out = nc.dram_tensor("dbg2_out", [P, 2048], F32, kind="ExternalOutput")

    x_t = x_in.ap().rearrange("(m p) n -> p m n", p=P)
    y_t = y_out.ap().rearrange("(m p) n -> p m n", p=P)

    with tile.TileContext(nc) as tc:
        with tc.tile_pool(name="dram", bufs=1, space="DRAM") as dram, \
             tc.tile_pool(name="const", bufs=1) as const:
            nc.gpsimd.load_library(mlp_lib)

            ident = const.tile([P, P], BF16)
            nc.sync.dma_start(ident[:], ident_in.ap())
            ones_s = const.tile([1, P], FP8)
            nc.sync.dma_start(ones_s[:], ones_in.ap())
            eps_t = const.tile([P, 1], F32)
            nc.sync.dma_start(eps_t[:], eps_in.ap())
            brow_s = const.tile([1, 4 * C], FP8)
            nc.sync.dma_start(brow_s[:], brows.ap())
            wvec_s = const.tile([32, C], BF16)
            nc.sync.dma_start(wvec_s[:], w_vec4.ap())
            b1c_s = const.tile([P, HID // P], F32)
            nc.sync.dma_start(b1c_s[:], b1_col.ap())

            x_all = const.tile([P, NW, C], F32)
            x2_all = const.tile([P, NW, C], BF16)
            zt_sbuf = const.tile([P, C // P, RS], FP8)
            q_sbuf = const.tile([P, NW, C], BF16)
            hT_sbuf = const.tile([P, HID // P, RS], FP8)

            kv_shard = dram.tile([RS, 2 * C], BF16)
            if "ag" not in _SKIP:
                kvt = dram.tile([L, 2 * C], BF16, addr_space="Shared")
            else:
                kvt = dram.tile([L, 2 * C], BF16)

            # ---------- LN helper: stats + normalized bf16 z (no affine) ----
            def ln_win(wk, ptp, src, m, copy_eng="v"):
                stats = wk.tile([P, 6], F32, tag="ln_st")
                nc.vector.bn_stats(stats[:], src)
                mv = wk.tile([P, 2], F32, tag="ln_mv")
                nc.vector.bn_aggr(mv[:], stats[:])
                sd = wk.tile([P, 1], F32, tag="ln_sd")
                nc.scalar.activation(sd[:], mv[:, 1:2], AF.Sqrt, bias=eps_t[:], scale=1.0)
                rs_ = wk.tile([P, 1], F32, tag="ln_rs")
                nc.vector.reciprocal(rs_[:], sd[:])
                nmr = wk.tile([P, 1], F32, tag="ln_nmr")
                nc.vector.scalar_tensor_tensor(
                    out=nmr[:], in0=mv[:, 0:1], scalar=-1.0, in1=rs_[:],
                    op0=ALU.mult, op1=ALU.mult)
                zf = wk.tile([P, C], BF16, tag="ln_zf")
                nc.scalar.activation(zf[:], src, AF.Identity, bias=nmr[:], scale=rs_[:])
                tp = ptp.tile([P, C], BF16, tag="tp")
                for c4 in range(C // P):
                    nc.tensor.transpose(tp[:, c4 * P:(c4 + 1) * P],
                                        zf[:, c4 * P:(c4 + 1) * P], ident[:])
                if copy_eng == "a":
                    nc.scalar.activation(
                        zt_sbuf[:, :, m * P:(m + 1) * P],
                        tp[:].rearrange("p (c4 q) -> p c4 q", q=P), AF.Identity)
                else:
                    nc.vector.tensor_copy(
                        zt_sbuf[:, :, m * P:(m + 1) * P],
                        tp[:].rearrange("p (c4 q) -> p c4 q", q=P))

            # ============ P1: LN1 + QKV ============
            if "p1" not in _SKIP:
             with tc.tile_pool(name="p1", bufs=2) as wk, \
                 tc.tile_pool(name="p1c", bufs=1) as cst, \
                 tc.tile_pool(name="p1tp", bufs=2, space="PSUM") as ptp, \
                 tc.tile_pool(name="p1ps", bufs=2, space="PSUM") as pps:
                wqkv_s = cst.tile([P, C // P, 3 * C], FP8, name="wqkv")
                nc.gpsimd.dma_start(wqkv_s[:], w_qkv.ap().rearrange("(ko p) n -> p ko n", p=P))
                kv_sh_t = kv_shard[:].rearrange("(m p) n -> p m n", p=P)
                for m in range(NW):
                    nc.sync.dma_start(x_all[:, m, :], x_t[:, m, :])
                    ln_win(wk, ptp, x_all[:, m, :], m, copy_eng="a")
                    for nb in range(3):
                        ps = pps.tile([P, C], F32, tag="ps")
                        for ko in range(0, C // P, 2):
                            nc.tensor.matmul(
                                ps[:],
                                lhsT=zt_sbuf[:, ko:ko + 2, m * P:(m + 1) * P],
                                rhs=wqkv_s[:, ko:ko + 2, nb * C:(nb + 1) * C],
                                start=(ko == 0), stop=(ko == C // P - 2 and nb != 0),
                                perf_mode=mybir.MatmulPerfMode.DoubleRow)
                        if nb == 0:
                            nc.tensor.matmul(ps[:], lhsT=ones_s[:], rhs=brow_s[0:1, 0:C],
                                             start=False, stop=True)
                            nc.scalar.activation(q_sbuf[:, m, :], ps[:], AF.Identity,
                                                 scale=1.0 / WS)
                        else:
                            kvb = wk.tile([P, C], BF16, tag="kvb")
                            # k stored negated so the edge phase can accumulate
                            # qe + (-k) on the PE via an identity matmul.
                            nc.vector.tensor_scalar_mul(
                                kvb[:], ps[:], (-1.0 if nb == 1 else 1.0) / WS)
                            nc.sync.dma_start(
                                kv_sh_t[:, m, (nb - 1) * C:nb * C], kvb[:])

            # ============ P2: AllGather ============
            if "ag" not in _SKIP:
                nc.gpsimd.collective_compute(
                    "AllGather", ALU.bypass, replica_groups=[list(range(NCORES))],
                    ins=[kv_shard[:].opt()], outs=[kvt[:].opt()])

            # ============ P3: edge windows + Wo + residual ============
            if "edge" not in _SKIP:
             with tc.tile_pool(name="big", bufs=2) as big, \
                 tc.tile_pool(name="ew", bufs=2) as wk, \
                 tc.tile_pool(name="pqe", bufs=1, space="PSUM") as pqe, \
                 tc.tile_pool(name="ppsW", bufs=1, space="PSUM") as ppsW, \
                 tc.tile_pool(name="ptpc", bufs=1, space="PSUM") as ptpc, \
                 tc.tile_pool(name="p5ps", bufs=2, space="PSUM") as p5ps, \
                 tc.tile_pool(name="woc", bufs=1) as woc:
                wo_s = woc.tile([P, C // P, C], FP8, name="wo")
                nc.gpsimd.dma_start(wo_s[:], w_o.ap().rearrange("(ko p) n -> p ko n", p=P))
                for w in range(NW):
                    idx_t = big.tile([P, TW // 16], I16, tag="idx", bufs=3)
                    nc.gpsimd.dma_start(idx_t[:], eidx.ap()[w])
                    kv_g = big.tile([P, NT, 2 * C], BF16, tag="kv", bufs=3)
                    nc.gpsimd.dma_gather(
                        out_ap=kv_g[:], in_ap=kvt[:], idxs_ap=idx_t[:],
                        num_idxs=TW, num_idxs_reg=TW, elem_size=2 * C,
                        single_packet=False)
                    s1_t = big.tile([P, NT, P], BF16, tag="s1")
                    nc.sync.dma_start(s1_t[:], s_e2r.ap()[w])
                    s2_t = big.tile([P, NT, P], BF16, tag="s2")
                    nc.sync.dma_start(s2_t[:], s_r2e.ap()[w])
                    rel_t = big.tile([P, NT, 4], BF16, tag="rel")
                    nc.sync.dma_start(rel_t[:], rel4.ap()[w])
                    bias_t = big.tile([P, NT, 8], F32, tag="bias")
                    nc.sync.dma_start(bias_t[:], biasC.ap()[w])

                    psW = ppsW.tile([P, 560], F32, tag="psW")
                    for t0 in range(0, NT, GS):
                        tb = min(GS, NT - t0)
                        qe = pqe.tile([P, GS, C], F32, tag="qe")
                        for d_ in range(tb):
                            nc.tensor.matmul(qe[:, d_, :], lhsT=s2_t[:, t0 + d_, :],
                                             rhs=q_sbuf[:, w, :], start=True, stop=False)
                            nc.tensor.matmul(qe[:, d_, :], lhsT=ident[:],
                                             rhs=kv_g[:, t0 + d_, 0:C],
                                             start=False, stop=True)
                        dsq = wk.tile([P, GS, C], BF16, tag="dsq")
                        nc.scalar.activation(dsq[:, 0:tb, :], qe[:, 0:tb, :], AF.Square)
                        s8 = wk.tile([P, GS, H], F32, tag="s8")
                        nc.vector.reduce_sum(
                            s8[:, 0:tb, :],
                            dsq[:, 0:tb, :].rearrange("p t (h d) -> p t h d", h=H),
                            axis=AX.X)
                        sc = wk.tile([P, GS, H], F32, tag="sc")
                        nc.vector.scalar_tensor_tensor(
                            out=sc[:, 0:tb, :], in0=s8[:, 0:tb, :], scalar=-inv_s,
                            in1=bias_t[:, t0:t0 + tb, :], op0=ALU.mult, op1=ALU.add)
                        eaux = wk.tile([P, GS, 48], BF16, tag="eaux")
                        nc.scalar.activation(
                            eaux[:, 0:tb, 0:16].rearrange("p t (h j) -> p t h j", h=H),
                            sc[:, 0:tb, :].unsqueeze(3).broadcast_to([P, tb, H, 2]),
                            AF.Exp)
                        exp2 = wk.tile([P, GS, 16], BF16, tag="exp2")
                        nc.scalar.activation(
                            exp2[:, 0:tb, :].rearrange("p t (h j) -> p t h j", h=H),
                            sc[:, 0:tb, :].unsqueeze(3).broadcast_to([P, tb, H, 2]),
                            AF.Exp)
                        if w == 0 and t0 == 0 and os.environ.get("KDBG") == "edge1":
                            d2 = wk.tile([P, 2048], F32, tag="d2")
                            nc.vector.tensor_copy(d2[:, 0:24], s8[:].rearrange("p t h -> p (t h)"))
                            nc.vector.tensor_copy(d2[:, 24:48], sc[:].rearrange("p t h -> p (t h)"))
                            nc.vector.tensor_copy(d2[:, 48:560], qe[:, 0, :])
                            nc.vector.tensor_copy(d2[:, 560:584], bias_t[:, 0:3, :].rearrange("p t h -> p (t h)"))
                            nc.sync.dma_start(dbg2_out.ap(), d2[:])
                        pev = wk.tile([P, GS, C], BF16, tag="pev")
                        for d_ in range(tb):
                            nc.vector.tensor_mul(
                                pev[:, d_, :].rearrange("p (h a j) -> p h a j", h=H, j=2),
                                exp2[:, d_, :].rearrange("p (h j) -> p h j", h=H)
                                    .unsqueeze(2).broadcast_to([P, H, DH // 2, 2]),
                                kv_g[:, t0 + d_, C:2 * C]
                                    .rearrange("p (h a j) -> p h a j", h=H, j=2))
                        nc.vector.tensor_mul(
                            eaux[:, 0:tb, 16:48].rearrange("p t (h a) -> p t h a", h=H),
                            eaux[:, 0:tb, 0:16].rearrange("p t (h j) -> p t h j", h=H)[:, :, :, 0:1]
                                .broadcast_to([P, tb, H, 4]),
                            rel_t[:, t0:t0 + tb, :].unsqueeze(2)
                                .broadcast_to([P, tb, H, 4]))
                        for d_ in range(tb):
                            t = t0 + d_
                            nc.tensor.matmul(psW[:, 0:512], lhsT=s1_t[:, t, :],
                                             rhs=pev[:, d_, :],
                                             start=(t == 0), stop=False)
                            nc.tensor.matmul(psW[:, 512:560], lhsT=s1_t[:, t, :],
                                             rhs=eaux[:, d_, :],
                                             start=(t == 0), stop=(t == NT - 1))

                    den = wk.tile([P, 16], F32, tag="den")
                    nc.vector.tensor_scalar_max(den[:], psW[:, 512:528], 1e-30)
                    rden = wk.tile([P, 16], F32, tag="rden")
                    nc.vector.reciprocal(rden[:], den[:])
                    # fold the (unnormalized) Wvec term into psW[0:512]: w_vec4 is
                    # head-block-diagonal, so the per-(row,head) rden factors
                    # through the sum.
                    anr = wk.tile([P, 32], BF16, tag="anr")
                    nc.scalar.activation(anr[:], psW[:, 528:560], AF.Identity)
                    tpc = ptpc.tile([P, C], BF16, tag="tpc")
                    nc.tensor.transpose(tpc[0:32, 0:P], anr[:], ident[:])
                    an_ts = wk.tile([32, P], BF16, tag="an_ts")
                    nc.scalar.activation(an_ts[:], tpc[0:32, 0:P], AF.Identity)
                    nc.tensor.matmul(psW[:, 0:512], lhsT=an_ts[:], rhs=wvec_s[:],
                                     start=False, stop=True)
                    attin = wk.tile([P, C], BF16, tag="attin")
                    nc.vector.tensor_mul(
                        attin[:].rearrange("p (h d) -> p h d", h=H),
                        psW[:, 0:512].rearrange("p (h d) -> p h d", h=H),
                        rden[:].rearrange("p (h j) -> p h j", h=H)[:, :, 0:1]
                            .broadcast_to([P, H, DH]))
                    tpa = ptpc.tile([P, C], BF16, tag="tpc")
                    for c4 in range(C // P):
                        nc.tensor.transpose(tpa[:, c4 * P:(c4 + 1) * P],
                                            attin[:, c4 * P:(c4 + 1) * P], ident[:])
                    at_sb = wk.tile([P, C // P, P], FP8, tag="at_sb")
                    nc.scalar.activation(
                        at_sb[:], tpa[:].rearrange("p (c4 q) -> p c4 q", q=P),
                        AF.Identity)
                    x2ps = p5ps.tile([P, C], F32, tag="p5")
                    for ko in range(0, C // P, 2):
                        nc.tensor.matmul(x2ps[:], lhsT=at_sb[:, ko:ko + 2, :],
                                         rhs=wo_s[:, ko:ko + 2, :],
                                         start=(ko == 0), stop=False,
                                         perf_mode=mybir.MatmulPerfMode.DoubleRow)
                    nc.tensor.matmul(x2ps[:], lhsT=ones_s[:], rhs=brow_s[0:1, C:2 * C],
                                     start=False, stop=True)
                    nc.vector.scalar_tensor_tensor(
                        out=x2_all[:, w, :], in0=x2ps[:], scalar=1.0 / WS,
                        in1=x_all[:, w, :], op0=ALU.mult, op1=ALU.add)

            # ============ P4: LN2 + FFN (fused, per row-chunk) ============
            if "p4" not in _SKIP:
             with tc.tile_pool(name="p4", bufs=2) as wk, \
                 tc.tile_pool(name="p4c", bufs=1) as cst, \
                 tc.tile_pool(name="p4tp", bufs=2, space="PSUM") as ptp, \
                 tc.tile_pool(name="f1ps", bufs=2, space="PSUM") as pps1, \
                 tc.tile_pool(name="f2ps", bufs=2, space="PSUM") as pps2:
                w1_s = cst.tile([P, C // P, HID], FP8, name="w1")
                nc.gpsimd.dma_start(w1_s[:], w_1.ap().rearrange("(ko p) n -> p ko n", p=P))
                w2_s = cst.tile([P, HID // P, C], FP8, name="w2")
                nc.gpsimd.dma_start(w2_s[:], w_2.ap().rearrange("(ko p) n -> p ko n", p=P))
                for rc in range(RS // 512):
                    for m in range(rc * 4, rc * 4 + 4):
                        ln_win(wk, ptp, x2_all[:, m, :], m)
                    for ht in range(HID // P):
                        ps = pps1.tile([P, 512], F32, tag="ps1")
                        for ko in range(0, C // P, 2):
                            nc.tensor.matmul(
                                ps[:], lhsT=w1_s[:, ko:ko + 2, ht * P:(ht + 1) * P],
                                rhs=zt_sbuf[:, ko:ko + 2, rc * 512:(rc + 1) * 512],
                                start=(ko == 0), stop=(ko == C // P - 2),
                                perf_mode=mybir.MatmulPerfMode.DoubleRow)
                        nc.scalar.activation(
                            hT_sbuf[:, ht, rc * 512:(rc + 1) * 512], ps[:],
                            AF.Gelu_apprx_tanh, bias=b1c_s[:, ht:ht + 1], scale=1.0 / WS)
                    for m in range(rc * 4, rc * 4 + 4):
                        ps = pps2.tile([P, C], F32, tag="ps2")
                        for ht in range(0, HID // P, 2):
                            nc.tensor.matmul(ps[:], lhsT=hT_sbuf[:, ht:ht + 2, m * P:(m + 1) * P],
                                             rhs=w2_s[:, ht:ht + 2, :],
                                             start=(ht == 0), stop=False,
                                             perf_mode=mybir.MatmulPerfMode.DoubleRow)
                        nc.tensor.matmul(ps[:], lhsT=ones_s[:], rhs=brow_s[0:1, 2 * C:3 * C],
                                         start=False, stop=True)
                        yt = wk.tile([P, C], F32, tag="y")
                        nc.vector.scalar_tensor_tensor(
                            out=yt[:], in0=ps[:], scalar=1.0 / WS, in1=x2_all[:, m, :],
                            op0=ALU.mult, op1=ALU.add)
                        nc.sync.dma_start(y_t[:, m, :], yt[:])

    nc.compile()
    return nc


def _prep(inputs):
    row = np.asarray(inputs["row_index"]).astype(np.int64).ravel()
    col = np.asarray(inputs["col_index"]).astype(np.int64).ravel()
    tcol = np.asarray(inputs["to_col_index"]).astype(np.int64).ravel()
    bias = np.asarray(inputs["pos_att_bias"], dtype=np.float32)
    dist = np.asarray(inputs["dist"], dtype=np.float32).ravel()
    pos = np.asarray(inputs["pos"], dtype=np.float32)
    cpos = np.asarray(inputs["col_pos"], dtype=np.float32)

    order = np.argsort(row, kind="stable")
    rs_, cs_, ts_ = row[order], col[order], tcol[order]
    win = rs_ // P
    counts = np.bincount(win, minlength=L // P)
    TW = int(np.ceil(max(int(counts.max()), 1) / P) * P)
    NT = TW // P
    starts = np.zeros(L // P + 1, np.int64)
    np.cumsum(counts, out=starts[1:])

    eidx_h = np.zeros((NCORES, NW, P, TW // 16), np.int16)
    rel4_h = np.zeros((NCORES, NW, P, NT, 4), np.float32)
    bias_h = np.full((NCORES, NW, P, NT, 8), -1e4, np.float32)
    s1_h = np.zeros((NCORES, NW, P, NT, P), np.float32)
    s2_h = np.zeros((NCORES, NW, P, NT, P), np.float32)

    for gw in range(L // P):
        c, w = divmod(gw, NW)
        s, e = int(starts[gw]), int(starts[gw + 1])
        n = e - s
        if n == 0:
            continue
        ecols = cs_[s:e]
        erows = (rs_[s:e] - gw * P).astype(np.int64)
        eo = order[s:e]
        j = np.arange(n)
        wrap = np.zeros((16, TW // 16), np.int16)
        wrap[j % 16, j // 16] = ecols.astype(np.int16)
        eidx_h[c, w] = np.tile(wrap, (8, 1))
        t_of = j // P
        e_of = j % P
        rel4_h[c, w, e_of, t_of, 0:3] = (cpos[ts_[s:e]] - pos[rs_[s:e]]) / dist[eo][:, None]
        rel4_h[c, w, e_of, t_of, 3] = 1.0
        bias_h[c, w, e_of, t_of, :] = bias[eo]
        s1_h[c, w, e_of, t_of, erows] = 1.0
        s2_h[c, w, erows, t_of, e_of] = 1.0

    import ml_dtypes
    bf = ml_dtypes.bfloat16
    return (TW, eidx_h, rel4_h.astype(bf), bias_h,
            s1_h.astype(bf), s2_h.astype(bf))


def kernel(**inputs):
    import ml_dtypes
    bf = ml_dtypes.bfloat16
    x = np.asarray(inputs["x"], dtype=np.float32)
    TW, eidx_h, rel4_h, bias_h, s1_h, s2_h = _prep(inputs)
    if TW not in _cache:
        _cache[TW] = _build(TW)
    nc = _cache[TW]

    f32 = lambda k: np.asarray(inputs[k], np.float32)
    g1, b1l = f32("ln1_g"), f32("ln1_b")
    g2, b2l = f32("ln2_g"), f32("ln2_b")
    Wq, Wk, Wv, Wo = f32("Wq"), f32("Wk"), f32("Wv"), f32("Wo")
    # Fold LN affine into the following matmuls; fold bk into bq (only the
    # difference q-k matters) and bv into bo (sum_e alpha = 1 per head).
    Wq_, Wk_, Wv_ = g1[:, None] * Wq, g1[:, None] * Wk, g1[:, None] * Wv
    bq_ = (b1l @ Wq + f32("bq")) - (b1l @ Wk + f32("bk"))
    bo_ = (b1l @ Wv + f32("bv")) @ Wo + f32("bo")
    W1_ = g2[:, None] * f32("W1")
    b1_ = b2l @ f32("W1") + f32("b1")
    import ml_dtypes as _md
    f8 = _md.float8_e4m3
    WS = 64.0
    w_qkv = (np.concatenate([Wq_, Wk_, Wv_], axis=1) * WS).astype(f8)

    wv4 = np.concatenate([f32("Wvec"), f32("bvec")[None, :]], axis=0)
    w_vec4 = np.zeros((32, C), np.float32)
    for h in range(H):
        w_vec4[4 * h:4 * h + 4, h * DH:(h + 1) * DH] = wv4[:, h * DH:(h + 1) * DH]

    brows = np.zeros((1, 4 * C), np.float32)
    brows[0, 0:C] = bq_
    brows[0, C:2 * C] = bo_
    brows[0, 2 * C:3 * C] = f32("b2")
    b1_col = np.ascontiguousarray(b1_.reshape(HID // P, P).T)

    in_maps = []
    for c in range(NCORES):
        in_maps.append(dict(
            x_in=np.ascontiguousarray(x[c * RS:(c + 1) * RS]),
            w_qkv=w_qkv, w_o=(Wo * WS).astype(f8),
            w_1=(W1_ * WS).astype(f8), w_2=(f32("W2") * WS).astype(f8),
            w_vec4=w_vec4.astype(bf), b1_col=b1_col,
            brows=(brows * WS).astype(f8),
            ident_in=np.eye(P, dtype=np.float32).astype(bf),
            ones_in=np.ones((1, P), np.float32).astype(f8),
            eps_in=np.full((P, 1), 1e-5, np.float32),
            eidx=eidx_h[c], rel4=rel4_h[c], biasC=bias_h[c],
            s_e2r=s1_h[c], s_r2e=s2_h[c],
        ))
    _last["nc"] = nc
    _last["in_maps"] = in_maps
    res = run_bass_kernel_spmd(nc, in_maps, list(range(NCORES)))
    global _last_res
    _last_res = res
    y = np.concatenate([res.results[c]["y_out"] for c in range(NCORES)], axis=0)
    return np.asarray(y, np.float32)


_last = {}
_last_res = None
